# revision 9
# baseline (speedup 1.0000x reference)
"""BoundaryLoss Trainium2 kernel (8-core data-parallel).

loss = mean( (softplus(x) - t*x) * w ),  w = 1 + 5*boundary(t > 0.5)
boundary = dilate2(m) & ~erode2(m), 3x3 cross SE, 2 iterations, zero pad.

Reformulation: two iterations of cross erosion/dilation equal one
erosion/dilation by the L1-diamond of radius 2 (13 cells).  With S = the
13-cell sum of the binary mask m (zero padded):
    eroded = [S == 13], dilated = [S >= 1], boundary = [1 <= S <= 12]
    w = 6 - r5,  r5 = 5*[S == 0 or S == 13] = relu(5|S - 6.5| - 27.5)
so, with s = softplus(x) - t*x and p = r5*s:
    sum(bce*w) = 6*(sum(sp) - sum(tx)) - sum(p)
summed per-partition, masked at the end by a per-strip-kind row-ownership
vector and reduced on the host (the cross-core all-reduce is 8x128 floats).

Per core: 4 images [1024,1024], split into 9 row-strips each (128 rows
loaded, owning 126/124/30 rows for the first/middle/tail strips; vertical
halo comes from the 2-row overlap, the top/bottom zero pad from band-matrix
truncation at partition edges).  Tiles pack two same-kind strips (an image
pair) side by side in the free dim (FD=2048).

Engines: S runs on the TensorEngine as 5 PSUM-accumulated band-matrix
matmuls per 512-col section (vertical reach via the band, horizontal reach
via column-shifted rhs windows, clipped at image edges = zero pad).
ScalarE does exp / ln(1+e) / |5S-32.5| from one activation-table set
(natural_log_exp_and_others), with the softplus row-sums riding the ln op's
accum_out.  VectorE does the mask threshold, t*x (with accum_out), two
bf16 tensor_tensor ops and one accumulating tensor_scalar.
"""

import numpy as np
import ml_dtypes

import concourse.bass as bass
import concourse.mybir as mybir
import concourse.tile as tile
from concourse.bass_utils import run_bass_kernel_spmd

F32 = mybir.dt.float32
BF16 = mybir.dt.bfloat16
ALU = mybir.AluOpType
ACT = mybir.ActivationFunctionType

N_CORES = 8
B, H, W = 32, 1024, 1024
B_LOC = B // N_CORES            # 4 images per core


# ---------------------------------------------------------------------------
# Workaround: the neuronxcc walrus build encodes at most one sync-wait per
# instruction; Tile attaches several.  Split them onto single-wait NOPs on
# the same engine right before the instruction (engines execute in order).
def _patched_drain_and_barrier(self, tick_clock, wait_clock):
    from bass_rust import ScopedClock

    nc = self.nc
    probe = nc.sync.nop(hint="tile_tail_wait_probe")
    wait_clock.add_sem_waits(probe.ins, ScopedClock({None: tick_clock.global_clock}))
    waits = list(probe.ins.sync_info.on_wait or [])
    if waits:
        probe.ins.sync_info = mybir.SyncInfo(on_wait=[waits[0]], on_update=[])
        for w in waits[1:]:
            n = nc.sync.nop(hint="tile_tail_wait_split", nofuse=True)
            n.ins.sync_info = mybir.SyncInfo(on_wait=[w], on_update=[])
    nc.sync.drain()
    nc.all_engine_barrier()
    assert self.sems is not None
    popped = nc._tile_sem_poison_stack.pop()
    assert popped is self._sem_poison
    nc.clear_and_free_semaphores(list(self.sems.allocated().values()))
    nc.all_engine_barrier()


tile.TileContext._drain_and_barrier = _patched_drain_and_barrier


def _split_multi_waits(nc: bass.Bass) -> None:
    seen = set()
    nidx = 0
    for ctx in nc.bb_map.values():
        bb = ctx.bb
        if id(bb) in seen:
            continue
        seen.add(id(bb))
        insts = bb.instructions
        i = 0
        while i < len(insts):
            inst = insts[i]
            si = inst.sync_info
            if si is not None and si.on_wait and len(si.on_wait) > 1:
                waits = list(si.on_wait)
                for w in waits[:-1]:
                    nop = mybir.InstNoOp(name=f"I-waitsplit-{nidx}", ins=[], outs=[])
                    nidx += 1
                    nop.engine = inst.engine
                    nop.sync_info = mybir.SyncInfo(on_wait=[w], on_update=[])
                    nc.register_instruction(nop)
                    insts.insert(i, nop)
                    i += 1
                inst.sync_info = mybir.SyncInfo(
                    on_wait=[waits[-1]], on_update=list(si.on_update or [])
                )
            i += 1
# ---------------------------------------------------------------------------


def _band(width: int) -> np.ndarray:
    k = np.arange(128)
    return (np.abs(k[:, None] - k[None, :]) <= width).astype(ml_dtypes.bfloat16)


def _own(lo: int, hi: int) -> np.ndarray:
    v = np.zeros((128, 1), dtype=np.float32)
    v[lo:hi] = 1
    return v


# jobs: (kind, load_row, img_pair) — two same-kind strips per tile.
# "s0": rows 0..127 loaded, owns rows 0..125 (top pad = band truncation)
# "mid": rows a..a+127 loaded, owns a+2..a+125 (a = 124k, k=1..7)
# "tail": rows 992..1023 loaded (32 real, rest zeroed), owns 994..1023
_JOBS = (
    [("s0", 0, p) for p in ((0, 1), (2, 3))]
    + [("mid", 124 * k, p) for p in ((0, 1), (2, 3)) for k in range(1, 8)]
    + [("tail", 992, p) for p in ((0, 1), (2, 3))]
)
_KIND_COLS = {"s0": (0, 2), "mid": (2, 16), "tail": (16, 18)}
_OWN_RANGES = {"s0": (0, 126), "mid": (2, 126), "tail": (2, 32)}


def build_nc(repeat: int = 1) -> bass.Bass:
    """repeat>1 wraps the tile loop in a HW For_i (timing variant)."""
    import contextlib

    nc = bass.Bass()

    x_d = nc.dram_tensor("inputs", [B_LOC, 1, H, W], F32, kind="ExternalInput")
    t_d = nc.dram_tensor("targets", [B_LOC, 1, H, W], F32, kind="ExternalInput")
    out_d = nc.dram_tensor("out", [128, 1], F32, kind="ExternalOutput")

    band_d = {w: nc.inline_tensor(_band(w), name=f"band{w}") for w in (0, 1, 2)}
    own_d = {k: nc.inline_tensor(_own(*r), name=f"own_{k}") for k, r in _OWN_RANGES.items()}

    n_jobs = len(_JOBS)
    terms = [(0, 2), (-1, 1), (1, 1), (-2, 0), (2, 0)]

    with tile.TileContext(nc) as tc:
        with (
            tc.tile_pool(name="const", bufs=1) as cpool,
            tc.tile_pool(name="acc", bufs=1) as apool,
            tc.tile_pool(name="work", bufs=3) as pool,
            tc.tile_pool(name="psum", bufs=2, space=bass.MemorySpace.PSUM) as psum,
        ):
            bands = {}
            for w in (0, 1, 2):
                bt = cpool.tile([128, 128], BF16, tag=f"band{w}")
                nc.sync.dma_start(bt[:], band_d[w][:])
                bands[w] = bt
            owns = {}
            for k, dten in own_d.items():
                ot = cpool.tile([128, 1], F32, tag=f"own_{k}")
                nc.sync.dma_start(ot[:], dten[:])
                owns[k] = ot
            bias_abs = cpool.tile([128, 1], F32, tag="bias_abs")
            nc.vector.memset(bias_abs[:], -32.5)

            acc_sp = apool.tile([128, n_jobs], F32, tag="acc_sp")
            acc_tx = apool.tile([128, n_jobs], F32, tag="acc_tx")
            acc_p = apool.tile([128, n_jobs], F32, tag="acc_p")
            nc.vector.memset(acc_sp[:], 0.0)
            nc.vector.memset(acc_tx[:], 0.0)
            nc.vector.memset(acc_p[:], 0.0)

            loop_ctx = tc.For_i(0, repeat, 1) if repeat > 1 else contextlib.nullcontext()
            with loop_ctx:
              for ti, (kind, row, pair) in enumerate(_JOBS):
                  t_t = pool.tile([128, 2 * W], F32, tag="t")
                  x_t = pool.tile([128, 2 * W], F32, tag="x")
                  m_t = pool.tile([128, 2 * W], BF16, tag="m")
                  a_t = pool.tile([128, 2 * W], BF16, tag="a")
                  r_t = pool.tile([128, 2 * W], BF16, tag="r")
                  e_t = pool.tile([128, 2 * W], BF16, tag="e")
                  sp_t = pool.tile([128, 2 * W], BF16, tag="sp")
                  tx_t = pool.tile([128, 2 * W], BF16, tag="tx")
                  s_t = pool.tile([128, 2 * W], BF16, tag="s")
                  p_t = pool.tile([128, 2 * W], BF16, tag="p")
                  s_ps = psum.tile([128, 2 * W], F32, tag="S")

                  nrows = 32 if kind == "tail" else 128
                  for h, img in enumerate(pair):
                      fc = h * W
                      if nrows < 128:
                          nc.vector.memset(t_t[:, fc : fc + W], 0.0)
                          nc.vector.memset(x_t[:, fc : fc + W], 0.0)
                      nc.sync.dma_start(
                          t_t[0:nrows, fc : fc + W], t_d[img, 0, row : row + nrows, :]
                      )
                      nc.sync.dma_start(
                          x_t[0:nrows, fc : fc + W], x_d[img, 0, row : row + nrows, :]
                      )

                  # binary mask, both halves in one dense op
                  nc.vector.tensor_scalar(m_t[:], t_t[:], 0.5, None, ALU.is_gt)

                  # S = diamond-2 sum: 5 band matmuls per 512-col section,
                  # windows clipped at image columns (= zero padding)
                  for sec in range(4):
                      hbase = (sec // 2) * W
                      o = (sec % 2) * 512
                      for i, (dj, wd) in enumerate(terms):
                          c0 = max(o + dj, 0)
                          c1 = min(o + dj + 512, W)
                          outp = s_ps[:, sec * 512 + c0 - o - dj : sec * 512 + c1 - o - dj]
                          nc.tensor.matmul(
                              outp,
                              bands[wd][:],
                              m_t[:, hbase + c0 : hbase + c1],
                              start=(i == 0),
                              stop=(i == len(terms) - 1),
                          )

                  # a = |5S - 32.5| ; r5 = relu(a - 27.5) = 5*[S==0 or S==13]
                  nc.scalar.activation(a_t[:], s_ps[:], ACT.Abs, bias=bias_abs[:], scale=5.0)
                  nc.vector.tensor_scalar(r_t[:], a_t[:], -27.5, 0.0, ALU.add, ALU.max)

                  # bce tail: sp = ln(1+e^x) (row-sums ride accum_out)
                  nc.scalar.activation(e_t[:], x_t[:], ACT.Exp)
                  nc.scalar.activation(
                      sp_t[:], e_t[:], ACT.Ln, bias=1.0,
                      accum_out=acc_sp[:, ti : ti + 1],
                  )
                  nc.vector.scalar_tensor_tensor(
                      tx_t[:], t_t[:], 1.0, x_t[:], ALU.mult, ALU.mult,
                      accum_out=acc_tx[:, ti : ti + 1],
                  )
                  nc.vector.tensor_tensor(s_t[:], sp_t[:], tx_t[:], ALU.subtract)
                  nc.vector.tensor_tensor(p_t[:], r_t[:], s_t[:], ALU.mult)
                  nc.vector.tensor_scalar(
                      p_t[:], p_t[:], 1.0, None, ALU.mult, ALU.add,
                      accum_out=acc_p[:, ti : ti + 1],
                  )

            # out = sum_kind own_k * (6*(sum sp - sum tx) - sum p)
            out_t = apool.tile([128, 1], F32, tag="out")
            first = True
            for kd in ("s0", "mid", "tail"):
                lo, hi = _KIND_COLS[kd]
                rsp = apool.tile([128, 1], F32, tag=f"rsp_{kd}")
                rtx = apool.tile([128, 1], F32, tag=f"rtx_{kd}")
                rp = apool.tile([128, 1], F32, tag=f"rp_{kd}")
                nc.vector.tensor_reduce(rsp[:], acc_sp[:, lo:hi], mybir.AxisListType.X, ALU.add)
                nc.vector.tensor_reduce(rtx[:], acc_tx[:, lo:hi], mybir.AxisListType.X, ALU.add)
                nc.vector.tensor_reduce(rp[:], acc_p[:, lo:hi], mybir.AxisListType.X, ALU.add)
                u_k = apool.tile([128, 1], F32, tag=f"u_{kd}")
                nc.vector.tensor_tensor(u_k[:], rsp[:], rtx[:], ALU.subtract)
                d_k = apool.tile([128, 1], F32, tag=f"d_{kd}")
                nc.vector.scalar_tensor_tensor(
                    d_k[:], u_k[:], 6.0, rp[:], ALU.mult, ALU.subtract
                )
                if first:
                    nc.vector.tensor_scalar(out_t[:], d_k[:], owns[kd][:], None, ALU.mult)
                    first = False
                else:
                    nc.vector.scalar_tensor_tensor(
                        out_t[:], d_k[:], owns[kd][:], out_t[:], ALU.mult, ALU.add
                    )
            nc.sync.dma_start(out_d[:], out_t[:])

    _split_multi_waits(nc)
    nc.finalize()
    return nc


_NC = None


def _get_nc():
    global _NC
    if _NC is None:
        _NC = build_nc()
    return _NC


def kernel(inputs: np.ndarray, targets: np.ndarray) -> np.ndarray:
    nc = _get_nc()
    in_maps = [
        {
            "inputs": np.ascontiguousarray(inputs[c * B_LOC : (c + 1) * B_LOC]),
            "targets": np.ascontiguousarray(targets[c * B_LOC : (c + 1) * B_LOC]),
        }
        for c in range(N_CORES)
    ]
    res = run_bass_kernel_spmd(nc, in_maps, list(range(N_CORES)))
    total = sum(float(r["out"].sum()) for r in res.results)
    return np.float32(total / (B * H * W))


# revision 10
# speedup vs baseline: 1.3839x; 1.3839x over previous
"""BoundaryLoss Trainium2 kernel (8-core data-parallel).

loss = mean( (softplus(x) - t*x) * w ),  w = 1 + 5*boundary(t > 0.5)
boundary = dilate2(m) & ~erode2(m), 3x3 cross SE, 2 iterations, zero pad.

Reformulation: two iterations of cross erosion/dilation equal one
erosion/dilation by the L1-diamond of radius 2 (13 cells).  With S = the
13-cell sum of the binary mask m (zero padded):
    eroded = [S == 13], dilated = [S >= 1], boundary = [1 <= S <= 12]
    w = 6 - r5,  r5 = 5*[S == 0 or S == 13] = relu(5|S - 6.5| - 27.5)
so, with s = softplus(x) - t*x and p = r5*s:
    sum(bce*w) = 6*(sum(sp) - sum(tx)) - sum(p)
summed per-partition, masked at the end by a per-strip-kind row-ownership
vector and reduced on the host (the cross-core all-reduce is 8x128 floats).

Per core: 4 images [1024,1024], split into 9 row-strips each (128 rows
loaded, owning 126/124/30 rows for the first/middle/tail strips; vertical
halo comes from the 2-row overlap, the top/bottom zero pad from band-matrix
truncation at partition edges).  Tiles pack two same-kind strips (an image
pair) side by side in the free dim (FD=2048).

Engines: S runs on the TensorEngine as 5 PSUM-accumulated band-matrix
matmuls per 512-col section (vertical reach via the band, horizontal reach
via column-shifted rhs windows, clipped at image edges = zero pad).
ScalarE does exp / ln(1+e) / |5S-32.5| from one activation-table set
(natural_log_exp_and_others), with the softplus row-sums riding the ln op's
accum_out.  VectorE does the mask threshold, t*x (with accum_out), two
bf16 tensor_tensor ops and one accumulating tensor_scalar.
"""

import numpy as np
import ml_dtypes

import concourse.bass as bass
import concourse.mybir as mybir
import concourse.tile as tile
from concourse.bass_utils import run_bass_kernel_spmd

F32 = mybir.dt.float32
BF16 = mybir.dt.bfloat16
ALU = mybir.AluOpType
ACT = mybir.ActivationFunctionType

N_CORES = 8
B, H, W = 32, 1024, 1024
B_LOC = B // N_CORES            # 4 images per core


# ---------------------------------------------------------------------------
# Workaround: the neuronxcc walrus build encodes at most one sync-wait per
# instruction; Tile attaches several.  Split them onto single-wait NOPs on
# the same engine right before the instruction (engines execute in order).
def _patched_drain_and_barrier(self, tick_clock, wait_clock):
    from bass_rust import ScopedClock

    nc = self.nc
    probe = nc.sync.nop(hint="tile_tail_wait_probe")
    wait_clock.add_sem_waits(probe.ins, ScopedClock({None: tick_clock.global_clock}))
    waits = list(probe.ins.sync_info.on_wait or [])
    if waits:
        probe.ins.sync_info = mybir.SyncInfo(on_wait=[waits[0]], on_update=[])
        for w in waits[1:]:
            n = nc.sync.nop(hint="tile_tail_wait_split", nofuse=True)
            n.ins.sync_info = mybir.SyncInfo(on_wait=[w], on_update=[])
    nc.sync.drain()
    nc.all_engine_barrier()
    assert self.sems is not None
    popped = nc._tile_sem_poison_stack.pop()
    assert popped is self._sem_poison
    nc.clear_and_free_semaphores(list(self.sems.allocated().values()))
    nc.all_engine_barrier()


tile.TileContext._drain_and_barrier = _patched_drain_and_barrier


def _split_multi_waits(nc: bass.Bass) -> None:
    seen = set()
    nidx = 0
    for ctx in nc.bb_map.values():
        bb = ctx.bb
        if id(bb) in seen:
            continue
        seen.add(id(bb))
        insts = bb.instructions
        i = 0
        while i < len(insts):
            inst = insts[i]
            si = inst.sync_info
            if si is not None and si.on_wait and len(si.on_wait) > 1:
                waits = list(si.on_wait)
                for w in waits[:-1]:
                    nop = mybir.InstNoOp(name=f"I-waitsplit-{nidx}", ins=[], outs=[])
                    nidx += 1
                    nop.engine = inst.engine
                    nop.sync_info = mybir.SyncInfo(on_wait=[w], on_update=[])
                    nc.register_instruction(nop)
                    insts.insert(i, nop)
                    i += 1
                inst.sync_info = mybir.SyncInfo(
                    on_wait=[waits[-1]], on_update=list(si.on_update or [])
                )
            i += 1
# ---------------------------------------------------------------------------


def _band(width: int) -> np.ndarray:
    k = np.arange(128)
    return (np.abs(k[:, None] - k[None, :]) <= width).astype(ml_dtypes.bfloat16)


def _own(lo: int, hi: int) -> np.ndarray:
    v = np.zeros((128, 1), dtype=np.float32)
    v[lo:hi] = 1
    return v


# jobs: (kind, load_row, img_pair) — two same-kind strips per tile.
# "s0": rows 0..127 loaded, owns rows 0..125 (top pad = band truncation)
# "mid": rows a..a+127 loaded, owns a+2..a+125 (a = 124k, k=1..7)
# "tail": rows 992..1023 loaded (32 real, rest zeroed), owns 994..1023
_JOBS = (
    [("s0", 0, p) for p in ((0, 1), (2, 3))]
    + [("mid", 124 * k, p) for p in ((0, 1), (2, 3)) for k in range(1, 8)]
    + [("tail", 992, p) for p in ((0, 1), (2, 3))]
)
_KIND_COLS = {"s0": (0, 2), "mid": (2, 16), "tail": (16, 18)}
_OWN_RANGES = {"s0": (0, 126), "mid": (2, 126), "tail": (2, 32)}


def build_nc(repeat: int = 1) -> bass.Bass:
    """repeat>1 wraps the tile loop in a HW For_i (timing variant)."""
    import contextlib

    nc = bass.Bass()

    x_d = nc.dram_tensor("inputs", [B_LOC, 1, H, W], F32, kind="ExternalInput")
    t_d = nc.dram_tensor("targets", [B_LOC, 1, H, W], F32, kind="ExternalInput")
    out_d = nc.dram_tensor("out", [128, 1], F32, kind="ExternalOutput")

    band_d = {w: nc.inline_tensor(_band(w), name=f"band{w}") for w in (0, 1, 2)}
    own_d = {k: nc.inline_tensor(_own(*r), name=f"own_{k}") for k, r in _OWN_RANGES.items()}

    n_jobs = len(_JOBS)
    terms = [(0, 2), (-1, 1), (1, 1), (-2, 0), (2, 0)]

    with tile.TileContext(nc) as tc:
        with (
            tc.tile_pool(name="const", bufs=1) as cpool,
            tc.tile_pool(name="acc", bufs=1) as apool,
            tc.tile_pool(name="work", bufs=3) as pool,
            tc.tile_pool(name="psum", bufs=2, space=bass.MemorySpace.PSUM) as psum,
        ):
            bands = {}
            for w in (0, 1, 2):
                bt = cpool.tile([128, 128], BF16, tag=f"band{w}")
                nc.sync.dma_start(bt[:], band_d[w][:])
                bands[w] = bt
            owns = {}
            for k, dten in own_d.items():
                ot = cpool.tile([128, 1], F32, tag=f"own_{k}")
                nc.sync.dma_start(ot[:], dten[:])
                owns[k] = ot
            bias_abs = cpool.tile([128, 1], F32, tag="bias_abs")
            nc.vector.memset(bias_abs[:], -32.5)

            acc_s = apool.tile([128, n_jobs], F32, tag="acc_s")
            acc_p = apool.tile([128, n_jobs], F32, tag="acc_p")
            nc.vector.memset(acc_s[:], 0.0)
            nc.vector.memset(acc_p[:], 0.0)

            loop_ctx = tc.For_i(0, repeat, 1) if repeat > 1 else contextlib.nullcontext()
            with loop_ctx:
              for ti, (kind, row, pair) in enumerate(_JOBS):
                  t_t = pool.tile([128, 2 * W], BF16, tag="t")
                  x_t = pool.tile([128, 2 * W], BF16, tag="x")
                  m_t = pool.tile([128, 2 * W], BF16, tag="m")
                  a_t = pool.tile([128, 2 * W], BF16, tag="a")
                  r_t = pool.tile([128, 2 * W], BF16, tag="r")
                  e_t = pool.tile([128, 2 * W], BF16, tag="e")
                  sp_t = pool.tile([128, 2 * W], BF16, tag="sp")
                  tx_t = pool.tile([128, 2 * W], BF16, tag="tx")
                  s_t = pool.tile([128, 2 * W], BF16, tag="s")
                  p_t = pool.tile([128, 2 * W], BF16, tag="p")
                  s_ps = psum.tile([128, 2 * W], F32, tag="S")

                  nrows = 32 if kind == "tail" else 128
                  for h, img in enumerate(pair):
                      fc = h * W
                      if nrows < 128:
                          nc.vector.memset(t_t[:, fc : fc + W], 0.0)
                          nc.vector.memset(x_t[:, fc : fc + W], 0.0)
                      nc.gpsimd.dma_start(
                          t_t[0:nrows, fc : fc + W], t_d[img, 0, row : row + nrows, :]
                      )
                      nc.gpsimd.dma_start(
                          x_t[0:nrows, fc : fc + W], x_d[img, 0, row : row + nrows, :]
                      )

                  # binary mask, both halves in one dense op
                  nc.vector.tensor_scalar(m_t[:], t_t[:], 0.5, None, ALU.is_gt)

                  # S = diamond-2 sum: 5 band matmuls per 512-col section,
                  # windows clipped at image columns (= zero padding)
                  for sec in range(4):
                      hbase = (sec // 2) * W
                      o = (sec % 2) * 512
                      for i, (dj, wd) in enumerate(terms):
                          c0 = max(o + dj, 0)
                          c1 = min(o + dj + 512, W)
                          outp = s_ps[:, sec * 512 + c0 - o - dj : sec * 512 + c1 - o - dj]
                          nc.tensor.matmul(
                              outp,
                              bands[wd][:],
                              m_t[:, hbase + c0 : hbase + c1],
                              start=(i == 0),
                              stop=(i == len(terms) - 1),
                          )

                  # a = |5S - 32.5| ; r5 = relu(a - 27.5) = 5*[S==0 or S==13]
                  nc.scalar.activation(a_t[:], s_ps[:], ACT.Abs, bias=bias_abs[:], scale=5.0)
                  nc.vector.tensor_scalar(r_t[:], a_t[:], -27.5, 0.0, ALU.add, ALU.max)

                  # bce tail: sp = ln(1+e^x) (row-sums ride accum_out)
                  nc.scalar.activation(e_t[:], x_t[:], ACT.Exp)
                  nc.scalar.activation(sp_t[:], e_t[:], ACT.Ln, bias=1.0)
                  nc.vector.tensor_tensor(tx_t[:], t_t[:], x_t[:], ALU.mult)
                  nc.vector.tensor_tensor(s_t[:], sp_t[:], tx_t[:], ALU.subtract)
                  nc.vector.tensor_scalar(
                      s_t[:], s_t[:], 1.0, None, ALU.mult, ALU.add,
                      accum_out=acc_s[:, ti : ti + 1],
                  )
                  nc.vector.tensor_tensor(p_t[:], r_t[:], s_t[:], ALU.mult)
                  nc.vector.tensor_scalar(
                      p_t[:], p_t[:], 1.0, None, ALU.mult, ALU.add,
                      accum_out=acc_p[:, ti : ti + 1],
                  )

            # out = sum_kind own_k * (6*(sum sp - sum tx) - sum p)
            out_t = apool.tile([128, 1], F32, tag="out")
            first = True
            for kd in ("s0", "mid", "tail"):
                lo, hi = _KIND_COLS[kd]
                rs = apool.tile([128, 1], F32, tag=f"rs_{kd}")
                rp = apool.tile([128, 1], F32, tag=f"rp_{kd}")
                nc.vector.tensor_reduce(rs[:], acc_s[:, lo:hi], mybir.AxisListType.X, ALU.add)
                nc.vector.tensor_reduce(rp[:], acc_p[:, lo:hi], mybir.AxisListType.X, ALU.add)
                d_k = apool.tile([128, 1], F32, tag=f"d_{kd}")
                nc.vector.scalar_tensor_tensor(
                    d_k[:], rs[:], 6.0, rp[:], ALU.mult, ALU.subtract
                )
                if first:
                    nc.vector.tensor_scalar(out_t[:], d_k[:], owns[kd][:], None, ALU.mult)
                    first = False
                else:
                    nc.vector.scalar_tensor_tensor(
                        out_t[:], d_k[:], owns[kd][:], out_t[:], ALU.mult, ALU.add
                    )
            nc.sync.dma_start(out_d[:], out_t[:])

    _split_multi_waits(nc)
    nc.finalize()
    return nc


_NC = None


def _get_nc():
    global _NC
    if _NC is None:
        _NC = build_nc()
    return _NC


def kernel(inputs: np.ndarray, targets: np.ndarray) -> np.ndarray:
    nc = _get_nc()
    in_maps = [
        {
            "inputs": np.ascontiguousarray(inputs[c * B_LOC : (c + 1) * B_LOC]),
            "targets": np.ascontiguousarray(targets[c * B_LOC : (c + 1) * B_LOC]),
        }
        for c in range(N_CORES)
    ]
    res = run_bass_kernel_spmd(nc, in_maps, list(range(N_CORES)))
    total = sum(float(r["out"].sum()) for r in res.results)
    return np.float32(total / (B * H * W))
